# revision 17
# baseline (speedup 1.0000x reference)
"""Trainium2 Bass kernel for nn_AutoregressiveFeedback (B=256 data-parallel / 8 cores).

Pipeline: MHA self-attention -> 3-layer LSTM warmup scan -> autoregressive
2-cell LSTM decode -> scaled dot-product attention over predictions -> projection.

Per-core layout strategy (Bc = 32):
  * attention:  scores folded through G_h = (Wq_h Wk_h^T)/sqrt(KD) and the
    value/output projection through P_h = Wv_h Wo_h (host-side, weight-only).
    Scores are built transposed (S^T[k,q]); exp on ScalarE; A@V runs in
    q-partition orientation with a ones column appended to v' so the softmax
    denominator lands in psum column 64 (per-partition -> cheap normalize);
    the normalized context is PE-transposed into xT form for the LSTM.
  * LSTM: z stays in [batch, gates] orientation.  The three layers run as a
    wavefront (layer l at tick tau handles t = tau - l); each gate chunk c
    occupies PE column-group c (4 chunks x 32 batch rows = 128 psum
    partitions).  Chunk c computes units S_c = [32c,32c+32) u [128+32c,+32)
    with gate columns host-permuted to [i f o | g] per chunk; the first 32
    unit-columns are k-tile-0 units, the next 32 are k-tile-1 units.  With
    that unit->column mapping, the hidden-state transpose h -> h^T is a
    single DVE stream-transpose (independent 32x32 blocks stay inside their
    partition quadrant), keeping the PE queue free of transposes.
  * decode: the linear feats() chain collapses to F' = Fw0 Fw1 Fw2, folded
    into cell-0's input weights (G = F' W0).  Cells 0/1 wavefront.  h1
    history is written straight into the pT archive that both the recurrence
    and the final attention read.
  * final attention: p p^T is symmetric so exp(scores) serves as its own
    transpose; the softmax denominator comes from activation accum_out.

All biases in this problem are zeros by construction (spec fill=zeros).
"""

import numpy as np
import ml_dtypes

import concourse.bass as bass
import concourse.bacc as bacc
import concourse.mybir as mybir
import concourse.tile as tile
from concourse.bass_utils import run_bass_kernel_spmd

BF = ml_dtypes.bfloat16
dt = mybir.dt
AF = mybir.ActivationFunctionType
ALU = mybir.AluOpType

B_FULL, FA, U, H, KD, NF = 256, 64, 256, 4, 64, 64
NCORES = 8
BC = B_FULL // NCORES  # 32


def _gate_perm(n_units):
    """Permute the 4*n_units gate columns: chunk c (256 cols) computes units
    S_c = [32c, 32c+32) u [128+32c, 128+32c+32), laid out [i f o | g] with
    unit order (ktile0 32 units, ktile1 32 units) inside each gate block."""
    i0, f0, g0, o0 = 0, n_units, 2 * n_units, 3 * n_units
    cols = []
    for c in range(0, 4):
        u = np.concatenate([np.arange(32 * c, 32 * c + 32),
                            np.arange(128 + 32 * c, 128 + 32 * c + 32)])
        cols.append(np.concatenate([i0 + u, f0 + u, g0 + u, o0 + u]))
    return np.concatenate(cols)


def _kt_split(w):
    """[K, N] -> [128, K//128, N] partition-major k-tiles."""
    K, N = w.shape
    assert K % 128 == 0
    return np.ascontiguousarray(w.reshape(K // 128, 128, N).transpose(1, 0, 2))


def build_host_tensors(inputs, T):
    f32 = np.float32
    g = lambda k: np.asarray(inputs[k], f32)
    Wq, Wk, Wv, Wo = g("Wq"), g("Wk"), g("Wv"), g("Wo")
    W0, U0, W1, U1, W2, U2 = g("W0"), g("U0"), g("W1"), g("U1"), g("W2"), g("U2")
    Fw0, Fw1, Fw2 = g("Fw0"), g("Fw1"), g("Fw2")
    pred_W = g("pred_W")
    x = g("inputs")
    ncores = x.shape[0] // BC

    gsb = np.zeros((128, 2, 64), f32)
    pcat = np.zeros((64, 256), f32)
    for h in range(H):
        Wq_h = Wq[:, h * KD:(h + 1) * KD]
        Wk_h = Wk[:, h * KD:(h + 1) * KD]
        Wv_h = Wv[:, h * KD:(h + 1) * KD]
        Wo_h = Wo[h * KD:(h + 1) * KD, :]
        G = (Wq_h @ Wk_h.T) / np.sqrt(KD)
        gsb[64 * (h % 2):64 * (h % 2) + 64, h // 2, :] = G
        pcat[:, h * 64:(h + 1) * 64] = Wv_h @ Wo_h
    pdup = np.concatenate([pcat, pcat], axis=0)

    permw = _gate_perm(U)
    # tanh(g) is computed as 2*sigmoid(2g)-1: double the g-gate columns so a
    # single sigmoid activation covers all four gates.
    gdbl = np.ones((4 * U,), f32)
    gdbl[2 * U:3 * U] = 2.0
    gd = gdbl[permw]
    W0rep = np.vstack([W0] * 4)
    wmov = [
        _kt_split(np.vstack([W0rep, U0])[:, permw] * gd),
        _kt_split(np.vstack([W1, U1])[:, permw] * gd),
        _kt_split(np.vstack([W2, U2])[:, permw] * gd),
    ]
    Fp = Fw0 @ Fw1 @ Fw2
    wdec = [
        _kt_split(np.vstack([Fp @ W0, U0])[:, permw] * gd),
        _kt_split(np.vstack([W1, U1])[:, permw] * gd),
    ]
    shared = {
        "gsb": gsb.astype(BF), "pdup": pdup.astype(BF),
        "wmov0": wmov[0].astype(BF), "wmov1": wmov[1].astype(BF),
        "wmov2": wmov[2].astype(BF),
        "wdec0": wdec[0].astype(BF), "wdec1": wdec[1].astype(BF),
        "predw": _kt_split(pred_W).astype(BF),
        "eye": np.eye(128, dtype=f32).astype(BF),
    }
    percore = []
    for c in range(ncores):
        xc = x[c * BC:(c + 1) * BC]
        inpT = np.ascontiguousarray(xc.transpose(2, 0, 1).reshape(FA, BC * T))
        percore.append({"inpT2": np.concatenate([inpT, inpT], 0).astype(BF)})
    return shared, percore


def build_program(T, S, attn_scale):
    BT = BC * T
    NT = BT // 128       # 128-row bt tiles
    KT = T // 128        # k tiles per sequence
    QT = T // 128
    nc = bacc.Bacc("TRN2", target_bir_lowering=False, debug=False)

    d_inpT2 = nc.dram_tensor("inpT2", [128, BT], dt.bfloat16, kind="ExternalInput")
    d_gsb = nc.dram_tensor("gsb", [128, 2, 64], dt.bfloat16, kind="ExternalInput")
    d_pdup = nc.dram_tensor("pdup", [128, 256], dt.bfloat16, kind="ExternalInput")
    d_wmov = [nc.dram_tensor(f"wmov{l}", [128, 4, 1024], dt.bfloat16,
                             kind="ExternalInput") for l in range(3)]
    d_wdec = [nc.dram_tensor(f"wdec{l}", [128, 4, 1024], dt.bfloat16,
                             kind="ExternalInput") for l in range(2)]
    d_predw = nc.dram_tensor("predw", [128, 2, 64], dt.bfloat16, kind="ExternalInput")
    d_eye = nc.dram_tensor("eye", [128, 128], dt.bfloat16, kind="ExternalInput")
    d_out = nc.dram_tensor("out", [BC, S, NF], dt.float32, kind="ExternalOutput")

    with tile.TileContext(nc) as tc:
        with tc.tile_pool(name="persist", bufs=1) as pp:
            eye_sb = pp.tile([128, 128], dt.bfloat16, tag="eye")
            nc.sync.dma_start(eye_sb[:], d_eye[:])
            predw_sb = pp.tile([128, 2, 64], dt.bfloat16, tag="predw")
            nc.sync.dma_start(predw_sb[:], d_predw[:])
            xT4a = pp.tile([128, BT], dt.bfloat16, tag="xT4a")
            xT4b = pp.tile([128, BT], dt.bfloat16, tag="xT4b")
            pT = pp.tile([128, S, 2, 32], dt.bfloat16, tag="pT")
            outf = pp.tile([S, BC * NF], dt.float32, tag="outf")

            # PE warm-up burst: ~5us of dense back-to-back matmuls so the
            # HAM clock-gate releases (K=8/8) before the real work starts;
            # runs while the input DMAs are still in flight.
            with (
                tc.tile_pool(name="warm_sb", bufs=1) as wsb,
                tc.tile_pool(name="warm_ps", bufs=1, space="PSUM") as wps,
            ):
                wscr = wsb.tile([128, 1], dt.float32, tag="wscr")
                wburst = wps.tile([128, 128], dt.float32, tag="wburst")
                for i in range(48):
                    nc.tensor.matmul(wburst[:], eye_sb[:], eye_sb[:, 0:128],
                                     skip_group_check=True)
                nc.vector.tensor_copy(wscr[:], wburst[:, 0:1])

            # ================= attention =================
            with (
                tc.tile_pool(name="attn_sb", bufs=1) as asb,
                tc.tile_pool(name="attn_roll", bufs=3) as arl,
            ):
                inpT2 = asb.tile([128, BT], dt.bfloat16, tag="inpT2")
                nc.sync.dma_start(inpT2[:], d_inpT2[:])
                gsb = asb.tile([128, 2, 64], dt.bfloat16, tag="gsb")
                nc.sync.dma_start(gsb[:], d_gsb[:])
                pdup = asb.tile([128, 256], dt.bfloat16, tag="pdup")
                nc.sync.dma_start(pdup[:], d_pdup[:])
                w1T = [asb.tile([128, BT], dt.bfloat16, tag=f"w1T{i}", name=f"w1T{i}")
                       for i in range(2)]
                vE = asb.tile([128, NT, 4, 65], dt.bfloat16, tag="vE")
                nc.vector.memset(vE[:, :, :, 64], 1.0)

                # stage A: w1T_h = G_h^T @ inpT ; v'4 = inp @ [P_0..P_3]
                with tc.tile_pool(name="attn_psA", bufs=2, space="PSUM") as apsA:
                    for ntile in range(BT // 512):
                        cols = slice(ntile * 512, ntile * 512 + 512)
                        ps = [apsA.tile([128, 512], dt.float32, tag=f"w1ps{j}", name=f"w1ps{j}")
                              for j in range(2)]
                        for h in range(H):
                            r = 64 * (h % 2)
                            nc.tensor.matmul(
                                ps[h // 2][r:r + 64, :],
                                gsb[r:r + 64, h // 2, :],
                                inpT2[r:r + 64, cols],
                                skip_group_check=True)
                        for i in range(2):
                            if ntile % 2 == 0:
                                nc.vector.tensor_copy(w1T[i][:, cols], ps[i][:])
                            else:
                                nc.scalar.copy(w1T[i][:, cols], ps[i][:])
                    for nt2 in range(NT):
                        r = 64 * (nt2 % 2)
                        ps = apsA.tile([128, 256], dt.float32, tag="vps", bufs=4)
                        nc.tensor.matmul(
                            ps[:], inpT2[r:r + 64, nt2 * 128:nt2 * 128 + 128],
                            pdup[r:r + 64, :])
                        src = ps[:].rearrange("p (h d) -> p h d", h=4)
                        if nt2 % 2 == 0:
                            nc.vector.tensor_copy(vE[:, nt2, :, 0:64], src)
                        else:
                            nc.scalar.copy(vE[:, nt2, :, 0:64], src)

                # per-batch attention
                with (
                    tc.tile_pool(name="attn_psB", bufs=1, space="PSUM") as apsB,
                    tc.tile_pool(name="attn_psT", bufs=2, space="PSUM") as apsT,
                ):
                    for b in range(BC):
                        STps = apsB.tile([128, H, KT, T], dt.float32, tag="STps")
                        for h in range(H):
                            r = 64 * (h % 2)
                            for kt in range(KT):
                                nc.tensor.matmul(
                                    STps[:, h, kt, :],
                                    inpT2[r:r + 64,
                                          b * T + kt * 128:b * T + kt * 128 + 128],
                                    w1T[h // 2][r:r + 64, b * T:b * T + T])
                        expTa = arl.tile([128, 2, KT, T], dt.bfloat16, tag="expTa")
                        expTb = arl.tile([128, 2, KT, T], dt.bfloat16, tag="expTb")
                        nc.scalar.activation(expTa[:], STps[:, 0:2, :, :], AF.Exp)
                        nc.scalar.activation(expTb[:], STps[:, 2:4, :, :], AF.Exp)
                        expT = {0: expTa, 1: expTb}
                        OPs = []
                        for qt in range(QT):
                            OP = apsB.tile([128, 4, 65], dt.float32, tag=f"OP{qt}")
                            OPs.append(OP)
                            n_mm = H * KT
                            i = 0
                            for h in range(H):
                                for kt in range(KT):
                                    nc.tensor.matmul(
                                        OP[:, h, :],
                                        expT[h // 2][:, h % 2, kt,
                                                     qt * 128:qt * 128 + 128],
                                        vE[:, b * KT + kt, h, :],
                                        start=(i == 0), stop=(i == n_mm - 1),
                                        skip_group_check=True)
                                    i += 1
                        rZ = arl.tile([128, QT, 4], dt.float32, tag="rZ")
                        x4 = [arl.tile([128, 256], dt.bfloat16, tag=f"x4_{qt}", name=f"x4_{qt}")
                              for qt in range(QT)]
                        for qt in range(QT):
                            nc.vector.reciprocal(
                                rZ[:, qt, :],
                                OPs[qt][:, :, 64])
                            zb = bass.AP(rZ.tensor, rZ[:, qt, :].offset,
                                         [rZ[:, qt, :].ap[0], [1, 4], [0, 64]])
                            nc.vector.tensor_tensor(
                                x4[qt][:].rearrange("p (h d) -> p h d", h=4),
                                OPs[qt][:, :, 0:64], zb, ALU.mult)
                        for fh, dstT in enumerate((xT4a, xT4b)):
                            tp = apsT.tile([128, QT * 128], dt.bfloat16, tag="xTps")
                            for qt in range(QT):
                                nc.tensor.transpose(
                                    tp[:, qt * 128:qt * 128 + 128],
                                    x4[qt][:, fh * 128:fh * 128 + 128],
                                    eye_sb[:, 0:128])
                            nc.vector.tensor_copy(dstT[:, b * T:b * T + T], tp[:])

            # ================= LSTM phases =================
            with (
                tc.tile_pool(name="lstm_state", bufs=1) as lst,
                tc.tile_pool(name="lstm_roll", bufs=4) as lrl,
                tc.tile_pool(name="lstm_ps", bufs=2, space="PSUM") as lps,
            ):
                wmov_sb = []
                for l in range(3):
                    w = lst.tile([128, 4, 1024], dt.bfloat16, tag=f"wmov{l}",
                                 name=f"wmov{l}")
                    nc.sync.dma_start(w[:], d_wmov[l][:])
                    wmov_sb.append(w)
                wdec_sb = []
                for l in range(2):
                    w = lst.tile([128, 4, 1024], dt.bfloat16, tag=f"wdec{l}",
                                 name=f"wdec{l}")
                    nc.sync.dma_start(w[:], d_wdec[l][:])
                    wdec_sb.append(w)
                cS = lst.tile([128, 3, 64], dt.float32, tag="cS")
                # double-buffered h^T state: [:, l, 0:32]=ktile0, [:, l, 32:64]=ktile1
                hTab = [lst.tile([128, 3, 64], dt.bfloat16, tag=f"hT{i}",
                                 name=f"hT{i}") for i in range(2)]
                nc.vector.memset(cS[:], 0.0)
                nc.vector.memset(hTab[0][:], 0.0)
                nc.vector.memset(hTab[1][:], 0.0)

                def cell_tick(Zp, Gs, T1, tcS, hS, slot, stats, rhss, hT_dst,
                              use_gpsimd=False, split_sigma=False):
                    # stats: list of 4 [128,32] stationary k-tiles
                    for kt in range(4):
                        for c in range(4):
                            nc.tensor.matmul(
                                Zp[32 * c:32 * c + 32, slot, 0:256],
                                stats[kt],
                                rhss[kt][:, 256 * c:256 * c + 256],
                                start=(kt == 0), stop=(kt == 3),
                                tile_position=(0, 32 * c),
                                skip_group_check=True)
                    # one sigmoid covers all gates; tanh(g) = 2*sigmoid(2g)-1
                    # (the 2x is folded into the g-gate weight columns).
                    # gate column order is [i f g | o]; with split_sigma the
                    # o-gate activation runs off the critical c-path.
                    if split_sigma:
                        nc.scalar.activation(Gs[:, slot, 0:192],
                                             Zp[:, slot, 0:192], AF.Sigmoid)
                        nc.scalar.activation(Gs[:, slot, 192:256],
                                             Zp[:, slot, 192:256], AF.Sigmoid)
                    else:
                        nc.scalar.activation(Gs[:, slot, 0:256],
                                             Zp[:, slot, 0:256], AF.Sigmoid)
                    nc.vector.tensor_scalar(T1[:, slot, :], Gs[:, slot, 128:192],
                                            2.0, -1.0, ALU.mult, ALU.add)
                    nc.vector.tensor_tensor(T1[:, slot, :], Gs[:, slot, 0:64],
                                            T1[:, slot, :], ALU.mult)
                    nc.vector.tensor_tensor(cS[:, slot, :], Gs[:, slot, 64:128],
                                            cS[:, slot, :], ALU.mult)
                    nc.vector.tensor_tensor(cS[:, slot, :], cS[:, slot, :],
                                            T1[:, slot, :], ALU.add)
                    nc.scalar.activation(tcS[:, slot, :], cS[:, slot, :], AF.Tanh)
                    eng2 = nc.gpsimd if use_gpsimd else nc.vector
                    eng2.tensor_tensor(hS[:, slot, :], Gs[:, slot, 192:256],
                                       tcS[:, slot, :], ALU.mult)
                    nc.vector.transpose(hT_dst, hS[:, slot, :])

                # ---- warmup: skew-2 wavefront (layer l handles t = tau - 2l,
                # so a cell's input comes from 2 ticks back and only the
                # self-recurrence is a 1-tick dependency) ----
                for tau in range(T + 4):
                    cur = hTab[tau % 2]
                    nxt = hTab[(tau + 1) % 2]
                    Zp = lps.tile([128, 3, 512], dt.float32, tag="Zp")
                    Gs = lrl.tile([128, 3, 256], dt.float32, tag="Gs")
                    T1 = lrl.tile([128, 3, 64], dt.float32, tag="T1w")
                    tcS = lrl.tile([128, 3, 64], dt.float32, tag="tcS")
                    hS = lrl.tile([128, 3, 64], dt.bfloat16, tag="hS")
                    for l in (2, 1, 0):
                        t = tau - 2 * l
                        if t < 0 or t >= T:
                            continue
                        wl = wmov_sb[l]
                        rhss = [wl[:, k, :] for k in range(4)]
                        if l == 0:
                            stats = [xT4a[:, t:BT:T], xT4b[:, t:BT:T],
                                     cur[:, 0, 0:32], cur[:, 0, 32:64]]
                        else:
                            # y_{l-1}(t) was produced at tick tau-2 -> lives
                            # in the buffer of parity (tau-1) == nxt
                            stats = [nxt[:, l - 1, 0:32], nxt[:, l - 1, 32:64],
                                     cur[:, l, 0:32], cur[:, l, 32:64]]
                        cell_tick(Zp, Gs, T1, tcS, hS, l, stats,
                                  rhss, nxt[:, l, :], use_gpsimd=True,
                                  split_sigma=True)

                # layer l processes t=T-1 at tau=T-1+2l, writing
                # hTab[(T+2l)%2] == hTab[T%2] for every layer.
                hfin = hTab[T % 2]
                h1fin = hfin
                nc.vector.tensor_copy(pT[:, 0, :, :], hfin[:, 2, :]
                                      .rearrange("p (k b) -> p k b", k=2))

                # ---- decode: 2-cell wavefront (emit cell1, cell0 per tick) ----
                # double-buffered h0 state
                hd = [lst.tile([128, 64], dt.bfloat16, tag=f"hd{i}",
                               name=f"hd{i}") for i in range(2)]
                # seed: cell0/cell1 carry states continue from warmup layers 0/1
                nc.vector.tensor_copy(hd[(0) % 2][:], hfin[:, 0, :])
                for tau in range(S):
                    cur = hd[tau % 2]
                    nxt = hd[(tau + 1) % 2]
                    Zp = lps.tile([128, 2, 512], dt.float32, tag="Zp")
                    Gs = lrl.tile([128, 2, 256], dt.float32, tag="Gsd")
                    T1 = lrl.tile([128, 2, 64], dt.float32, tag="T1d")
                    tcS = lrl.tile([128, 2, 64], dt.float32, tag="tcSd")
                    hS = lrl.tile([128, 2, 64], dt.bfloat16, tag="hSd")
                    w1_ = tau           # cell1 computes step w1_
                    if 1 <= w1_ <= S - 1:
                        if w1_ == 1:
                            h1prev = [h1fin[:, 1, 0:32], h1fin[:, 1, 32:64]]
                        else:
                            h1prev = [pT[:, w1_ - 1, 0, :], pT[:, w1_ - 1, 1, :]]
                        # h1prev is ready a step earlier than h0 -> put it
                        # first so its matmuls (with start=True) run early
                        stats = h1prev + [cur[:, 0:32], cur[:, 32:64]]
                        rhss = [wdec_sb[1][:, k, :] for k in (2, 3, 0, 1)]
                        cell_tick(Zp, Gs, T1, tcS, hS, 1, stats,
                                  rhss, pT[:, w1_, :, :].rearrange("p k b -> p (k b)"),
                                  split_sigma=True)
                    w0 = tau + 1        # cell0 computes step w0
                    if w0 <= S - 1:
                        stats = [cur[:, 0:32], cur[:, 32:64],
                                 pT[:, w0 - 1, 0, :], pT[:, w0 - 1, 1, :]]
                        rhss = [wdec_sb[0][:, k, :] for k in (2, 3, 0, 1)]
                        cell_tick(Zp, Gs, T1, tcS, hS, 0, stats,
                                  rhss, nxt[:], split_sigma=True)

            # ================= final attention over p =================
            with (
                tc.tile_pool(name="fin_roll", bufs=4) as frl,
                tc.tile_pool(name="fin_ps", bufs=2, space="PSUM") as fps,
            ):
                for b in range(BC):
                    ppps = fps.tile([S, 64], dt.float32, tag="ppps")
                    s2ps = fps.tile([S, S], dt.float32, tag="s2ps")
                    for kt in range(2):
                        pslice = pT[:, :, kt, b]   # [128, S] stride 64
                        nc.tensor.matmul(ppps[:], pslice, predw_sb[:, kt, :],
                                         start=(kt == 0), stop=(kt == 1))
                        nc.tensor.matmul(s2ps[:], pslice, pslice,
                                         start=(kt == 0), stop=(kt == 1))
                    expw = frl.tile([S, S], dt.bfloat16, tag="expw")
                    z2 = frl.tile([S, 1], dt.float32, tag="z2")
                    nc.scalar.activation(expw[:], s2ps[:], AF.Exp,
                                         scale=float(attn_scale),
                                         accum_out=z2[:])
                    ppsb = frl.tile([S, 64], dt.bfloat16, tag="ppsb")
                    nc.vector.tensor_copy(ppsb[:], ppps[:])
                    ops = fps.tile([S, 64], dt.float32, tag="ops")
                    nc.tensor.matmul(ops[:], expw[:], ppsb[:])
                    rz2 = frl.tile([S, 1], dt.float32, tag="rz2")
                    nc.vector.reciprocal(rz2[:], z2[:])
                    nc.vector.tensor_scalar(outf[:, b * NF:(b + 1) * NF], ops[:],
                                            rz2[:], None, ALU.mult)
                nc.sync.dma_start(
                    d_out[:].rearrange("b s f -> s b f"),
                    outf[:].rearrange("s (b f) -> s b f", b=BC))

    nc.compile()
    return nc


_cache = {}


def kernel(**inputs):
    x = np.asarray(inputs["inputs"])
    T = x.shape[1]
    S = 64
    attn_scale = float(np.asarray(inputs["attn_scale"]))
    ncores = x.shape[0] // BC

    shared, percore = build_host_tensors(inputs, T)
    key = (T, S, round(attn_scale, 9))
    if key not in _cache:
        _cache[key] = build_program(T, S, attn_scale)
    nc = _cache[key]

    in_maps = [dict(shared, **percore[c]) for c in range(ncores)]
    res = run_bass_kernel_spmd(nc, in_maps, list(range(ncores)))
    out = np.concatenate([res.results[c]["out"] for c in range(ncores)], axis=0)
    return np.ascontiguousarray(out.astype(np.float32))


# revision 18
# speedup vs baseline: 1.0872x; 1.0872x over previous
"""Trainium2 Bass kernel for nn_AutoregressiveFeedback (B=256 data-parallel / 8 cores).

Pipeline: MHA self-attention -> 3-layer LSTM warmup scan -> autoregressive
2-cell LSTM decode -> scaled dot-product attention over predictions -> projection.

Per-core layout strategy (Bc = 32):
  * attention:  scores folded through G_h = (Wq_h Wk_h^T)/sqrt(KD) and the
    value/output projection through P_h = Wv_h Wo_h (host-side, weight-only).
    Scores are built transposed (S^T[k,q]); exp on ScalarE; A@V runs in
    q-partition orientation with a ones column appended to v' so the softmax
    denominator lands in psum column 64 (per-partition -> cheap normalize);
    the normalized context is PE-transposed into xT form for the LSTM.
  * LSTM: z stays in [batch, gates] orientation.  The three layers run as a
    skew-2 wavefront (layer l at tick tau handles t = tau - 2l, so a cell's
    input arrives two ticks early and only the self-recurrence is a one-tick
    dependency); each gate chunk c
    occupies PE column-group c (4 chunks x 32 batch rows = 128 psum
    partitions).  Chunk c computes units S_c = [32c,32c+32) u [128+32c,+32)
    with gate columns host-permuted to [i f g | o] per chunk; the first 32
    unit-columns are k-tile-0 units, the next 32 are k-tile-1 units.  With
    that unit->column mapping, the hidden-state transpose h -> h^T is a
    single DVE stream-transpose (independent 32x32 blocks stay inside their
    partition quadrant), keeping the PE queue free of transposes.
  * decode: the linear feats() chain collapses to F' = Fw0 Fw1 Fw2, folded
    into cell-0's input weights (G = F' W0).  Cells 0/1 wavefront.  h1
    history is written straight into the pT archive that both the recurrence
    and the final attention read.
  * final attention: p p^T is symmetric so exp(scores) serves as its own
    transpose; the softmax denominator comes from activation accum_out.

All biases in this problem are zeros by construction (spec fill=zeros).
"""

import numpy as np
import ml_dtypes

import concourse.bass as bass
import concourse.bacc as bacc
import concourse.mybir as mybir
import concourse.tile as tile
from concourse.bass_utils import run_bass_kernel_spmd

BF = ml_dtypes.bfloat16
dt = mybir.dt
AF = mybir.ActivationFunctionType
ALU = mybir.AluOpType

B_FULL, FA, U, H, KD, NF = 256, 64, 256, 4, 64, 64
NCORES = 8
BC = B_FULL // NCORES  # 32


def _gate_perm(n_units):
    """Permute the 4*n_units gate columns: chunk c (256 cols) computes units
    S_c = [32c, 32c+32) u [128+32c, 128+32c+32), laid out [i f o | g] with
    unit order (ktile0 32 units, ktile1 32 units) inside each gate block."""
    i0, f0, g0, o0 = 0, n_units, 2 * n_units, 3 * n_units
    cols = []
    for c in range(0, 4):
        u = np.concatenate([np.arange(32 * c, 32 * c + 32),
                            np.arange(128 + 32 * c, 128 + 32 * c + 32)])
        cols.append(np.concatenate([i0 + u, f0 + u, g0 + u, o0 + u]))
    return np.concatenate(cols)


def _kt_split(w):
    """[K, N] -> [128, K//128, N] partition-major k-tiles."""
    K, N = w.shape
    assert K % 128 == 0
    return np.ascontiguousarray(w.reshape(K // 128, 128, N).transpose(1, 0, 2))


def build_host_tensors(inputs, T):
    f32 = np.float32
    g = lambda k: np.asarray(inputs[k], f32)
    Wq, Wk, Wv, Wo = g("Wq"), g("Wk"), g("Wv"), g("Wo")
    W0, U0, W1, U1, W2, U2 = g("W0"), g("U0"), g("W1"), g("U1"), g("W2"), g("U2")
    Fw0, Fw1, Fw2 = g("Fw0"), g("Fw1"), g("Fw2")
    pred_W = g("pred_W")
    x = g("inputs")
    ncores = x.shape[0] // BC

    gsb = np.zeros((128, 2, 64), f32)
    pcat = np.zeros((64, 256), f32)
    for h in range(H):
        Wq_h = Wq[:, h * KD:(h + 1) * KD]
        Wk_h = Wk[:, h * KD:(h + 1) * KD]
        Wv_h = Wv[:, h * KD:(h + 1) * KD]
        Wo_h = Wo[h * KD:(h + 1) * KD, :]
        G = (Wq_h @ Wk_h.T) / np.sqrt(KD)
        gsb[64 * (h % 2):64 * (h % 2) + 64, h // 2, :] = G
        pcat[:, h * 64:(h + 1) * 64] = Wv_h @ Wo_h
    pdup = np.concatenate([pcat, pcat], axis=0)

    permw = _gate_perm(U)
    # tanh(g) is computed as 2*sigmoid(2g)-1: double the g-gate columns so a
    # single sigmoid activation covers all four gates.
    gdbl = np.ones((4 * U,), f32)
    gdbl[2 * U:3 * U] = 2.0
    gd = gdbl[permw]
    W0rep = np.vstack([W0] * 4)
    wmov = [
        _kt_split(np.vstack([W0rep, U0])[:, permw] * gd),
        _kt_split(np.vstack([W1, U1])[:, permw] * gd),
        _kt_split(np.vstack([W2, U2])[:, permw] * gd),
    ]
    Fp = Fw0 @ Fw1 @ Fw2
    wdec = [
        _kt_split(np.vstack([Fp @ W0, U0])[:, permw] * gd),
        _kt_split(np.vstack([W1, U1])[:, permw] * gd),
    ]
    shared = {
        "gsb": gsb.astype(BF), "pdup": pdup.astype(BF),
        "wmov0": wmov[0].astype(BF), "wmov1": wmov[1].astype(BF),
        "wmov2": wmov[2].astype(BF),
        "wdec0": wdec[0].astype(BF), "wdec1": wdec[1].astype(BF),
        "predw": _kt_split(pred_W).astype(BF),
        "eye": np.eye(128, dtype=f32).astype(BF),
    }
    percore = []
    for c in range(ncores):
        xc = x[c * BC:(c + 1) * BC]
        inpT = np.ascontiguousarray(xc.transpose(2, 0, 1).reshape(FA, BC * T))
        percore.append({"inpT2": np.concatenate([inpT, inpT], 0).astype(BF)})
    return shared, percore


def build_program(T, S, attn_scale):
    BT = BC * T
    NT = BT // 128       # 128-row bt tiles
    KT = T // 128        # k tiles per sequence
    QT = T // 128
    nc = bacc.Bacc("TRN2", target_bir_lowering=False, debug=False)

    d_inpT2 = nc.dram_tensor("inpT2", [128, BT], dt.bfloat16, kind="ExternalInput")
    d_gsb = nc.dram_tensor("gsb", [128, 2, 64], dt.bfloat16, kind="ExternalInput")
    d_pdup = nc.dram_tensor("pdup", [128, 256], dt.bfloat16, kind="ExternalInput")
    d_wmov = [nc.dram_tensor(f"wmov{l}", [128, 4, 1024], dt.bfloat16,
                             kind="ExternalInput") for l in range(3)]
    d_wdec = [nc.dram_tensor(f"wdec{l}", [128, 4, 1024], dt.bfloat16,
                             kind="ExternalInput") for l in range(2)]
    d_predw = nc.dram_tensor("predw", [128, 2, 64], dt.bfloat16, kind="ExternalInput")
    d_eye = nc.dram_tensor("eye", [128, 128], dt.bfloat16, kind="ExternalInput")
    d_out = nc.dram_tensor("out", [BC, S, NF], dt.float32, kind="ExternalOutput")

    with tile.TileContext(nc) as tc:
        with tc.tile_pool(name="persist", bufs=1) as pp:
            eye_sb = pp.tile([128, 128], dt.bfloat16, tag="eye")
            nc.sync.dma_start(eye_sb[:], d_eye[:])
            predw_sb = pp.tile([128, 2, 64], dt.bfloat16, tag="predw")
            nc.sync.dma_start(predw_sb[:], d_predw[:])
            xT4a = pp.tile([128, BT], dt.bfloat16, tag="xT4a")
            xT4b = pp.tile([128, BT], dt.bfloat16, tag="xT4b")
            pT = pp.tile([128, S, 2, 32], dt.bfloat16, tag="pT")
            outf = pp.tile([S, BC * NF], dt.float32, tag="outf")

            # PE warm-up burst: ~5us of dense back-to-back matmuls so the
            # HAM clock-gate releases (K=8/8) before the real work starts;
            # runs while the input DMAs are still in flight.
            with (
                tc.tile_pool(name="warm_sb", bufs=1) as wsb,
                tc.tile_pool(name="warm_ps", bufs=1, space="PSUM") as wps,
            ):
                wscr = wsb.tile([128, 1], dt.float32, tag="wscr")
                wburst = wps.tile([128, 128], dt.float32, tag="wburst")
                for i in range(48):
                    nc.tensor.matmul(wburst[:], eye_sb[:], eye_sb[:, 0:128],
                                     skip_group_check=True)
                nc.vector.tensor_copy(wscr[:], wburst[:, 0:1])

            # ================= attention =================
            with (
                tc.tile_pool(name="attn_sb", bufs=1) as asb,
                tc.tile_pool(name="attn_roll", bufs=3) as arl,
            ):
                inpT2 = asb.tile([128, BT], dt.bfloat16, tag="inpT2")
                nc.sync.dma_start(inpT2[:], d_inpT2[:])
                gsb = asb.tile([128, 2, 64], dt.bfloat16, tag="gsb")
                nc.sync.dma_start(gsb[:], d_gsb[:])
                pdup = asb.tile([128, 256], dt.bfloat16, tag="pdup")
                nc.sync.dma_start(pdup[:], d_pdup[:])
                w1T = [asb.tile([128, BT], dt.bfloat16, tag=f"w1T{i}", name=f"w1T{i}")
                       for i in range(2)]
                vE = asb.tile([128, NT, 4, 65], dt.bfloat16, tag="vE")
                nc.vector.memset(vE[:, :, :, 64], 1.0)

                # stage A: w1T_h = G_h^T @ inpT ; v'4 = inp @ [P_0..P_3]
                with tc.tile_pool(name="attn_psA", bufs=2, space="PSUM") as apsA:
                    for ntile in range(BT // 512):
                        cols = slice(ntile * 512, ntile * 512 + 512)
                        ps = [apsA.tile([128, 512], dt.float32, tag=f"w1ps{j}", name=f"w1ps{j}")
                              for j in range(2)]
                        for h in range(H):
                            r = 64 * (h % 2)
                            nc.tensor.matmul(
                                ps[h // 2][r:r + 64, :],
                                gsb[r:r + 64, h // 2, :],
                                inpT2[r:r + 64, cols],
                                skip_group_check=True)
                        for i in range(2):
                            if ntile % 2 == 0:
                                nc.vector.tensor_copy(w1T[i][:, cols], ps[i][:])
                            else:
                                nc.scalar.copy(w1T[i][:, cols], ps[i][:])
                    for nt2 in range(NT):
                        r = 64 * (nt2 % 2)
                        ps = apsA.tile([128, 256], dt.float32, tag="vps", bufs=4)
                        nc.tensor.matmul(
                            ps[:], inpT2[r:r + 64, nt2 * 128:nt2 * 128 + 128],
                            pdup[r:r + 64, :])
                        src = ps[:].rearrange("p (h d) -> p h d", h=4)
                        if nt2 % 2 == 0:
                            nc.vector.tensor_copy(vE[:, nt2, :, 0:64], src)
                        else:
                            nc.scalar.copy(vE[:, nt2, :, 0:64], src)

                # per-batch attention
                with (
                    tc.tile_pool(name="attn_psB", bufs=1, space="PSUM") as apsB,
                    tc.tile_pool(name="attn_psT", bufs=2, space="PSUM") as apsT,
                ):
                    for b in range(BC):
                        STps = apsB.tile([128, H, KT, T], dt.float32, tag="STps")
                        for h in range(H):
                            r = 64 * (h % 2)
                            for kt in range(KT):
                                nc.tensor.matmul(
                                    STps[:, h, kt, :],
                                    inpT2[r:r + 64,
                                          b * T + kt * 128:b * T + kt * 128 + 128],
                                    w1T[h // 2][r:r + 64, b * T:b * T + T])
                        expTa = arl.tile([128, 2, KT, T], dt.bfloat16, tag="expTa")
                        expTb = arl.tile([128, 2, KT, T], dt.bfloat16, tag="expTb")
                        nc.scalar.activation(expTa[:], STps[:, 0:2, :, :], AF.Exp)
                        nc.scalar.activation(expTb[:], STps[:, 2:4, :, :], AF.Exp)
                        expT = {0: expTa, 1: expTb}
                        OPs = []
                        for qt in range(QT):
                            OP = apsB.tile([128, 4, 65], dt.float32, tag=f"OP{qt}")
                            OPs.append(OP)
                            n_mm = H * KT
                            i = 0
                            for h in range(H):
                                for kt in range(KT):
                                    nc.tensor.matmul(
                                        OP[:, h, :],
                                        expT[h // 2][:, h % 2, kt,
                                                     qt * 128:qt * 128 + 128],
                                        vE[:, b * KT + kt, h, :],
                                        start=(i == 0), stop=(i == n_mm - 1),
                                        skip_group_check=True)
                                    i += 1
                        rZ = arl.tile([128, QT, 4], dt.float32, tag="rZ")
                        x4 = [arl.tile([128, 256], dt.bfloat16, tag=f"x4_{qt}", name=f"x4_{qt}")
                              for qt in range(QT)]
                        for qt in range(QT):
                            nc.vector.reciprocal(
                                rZ[:, qt, :],
                                OPs[qt][:, :, 64])
                            zb = bass.AP(rZ.tensor, rZ[:, qt, :].offset,
                                         [rZ[:, qt, :].ap[0], [1, 4], [0, 64]])
                            nc.vector.tensor_tensor(
                                x4[qt][:].rearrange("p (h d) -> p h d", h=4),
                                OPs[qt][:, :, 0:64], zb, ALU.mult)
                        for fh, dstT in enumerate((xT4a, xT4b)):
                            tp = apsT.tile([128, QT * 128], dt.bfloat16, tag="xTps")
                            for qt in range(QT):
                                nc.tensor.transpose(
                                    tp[:, qt * 128:qt * 128 + 128],
                                    x4[qt][:, fh * 128:fh * 128 + 128],
                                    eye_sb[:, 0:128])
                            nc.vector.tensor_copy(dstT[:, b * T:b * T + T], tp[:])

            # ================= LSTM phases =================
            with (
                tc.tile_pool(name="lstm_state", bufs=1) as lst,
                tc.tile_pool(name="lstm_roll", bufs=4) as lrl,
                tc.tile_pool(name="lstm_ps", bufs=2, space="PSUM") as lps,
            ):
                wmov_sb = []
                for l in range(3):
                    w = lst.tile([128, 4, 1024], dt.bfloat16, tag=f"wmov{l}",
                                 name=f"wmov{l}")
                    nc.sync.dma_start(w[:], d_wmov[l][:])
                    wmov_sb.append(w)
                wdec_sb = []
                for l in range(2):
                    w = lst.tile([128, 4, 1024], dt.bfloat16, tag=f"wdec{l}",
                                 name=f"wdec{l}")
                    nc.sync.dma_start(w[:], d_wdec[l][:])
                    wdec_sb.append(w)
                cS = lst.tile([128, 3, 64], dt.float32, tag="cS")
                # double-buffered h^T state: [:, l, 0:32]=ktile0, [:, l, 32:64]=ktile1
                hTab = [lst.tile([128, 3, 64], dt.bfloat16, tag=f"hT{i}",
                                 name=f"hT{i}") for i in range(2)]
                nc.vector.memset(cS[:], 0.0)
                nc.vector.memset(hTab[0][:], 0.0)
                nc.vector.memset(hTab[1][:], 0.0)

                def cell_tick(Zp, Gs, T1, tcS, hS, slot, stats, rhss, hT_dst,
                              use_gpsimd=False, split_sigma=False):
                    # stats: list of 4 [128,32] stationary k-tiles
                    for kt in range(4):
                        for c in range(4):
                            nc.tensor.matmul(
                                Zp[32 * c:32 * c + 32, slot, 0:256],
                                stats[kt],
                                rhss[kt][:, 256 * c:256 * c + 256],
                                start=(kt == 0), stop=(kt == 3),
                                tile_position=(0, 32 * c),
                                skip_group_check=True)
                    # one sigmoid covers all gates; tanh(g) = 2*sigmoid(2g)-1
                    # (the 2x is folded into the g-gate weight columns).
                    # gate column order is [i f g | o]; with split_sigma the
                    # o-gate activation runs off the critical c-path.
                    if split_sigma:
                        nc.scalar.activation(Gs[:, slot, 0:192],
                                             Zp[:, slot, 0:192], AF.Sigmoid)
                        nc.scalar.activation(Gs[:, slot, 192:256],
                                             Zp[:, slot, 192:256], AF.Sigmoid)
                    else:
                        nc.scalar.activation(Gs[:, slot, 0:256],
                                             Zp[:, slot, 0:256], AF.Sigmoid)
                    nc.vector.tensor_scalar(T1[:, slot, :], Gs[:, slot, 128:192],
                                            2.0, -1.0, ALU.mult, ALU.add)
                    nc.vector.tensor_tensor(T1[:, slot, :], Gs[:, slot, 0:64],
                                            T1[:, slot, :], ALU.mult)
                    nc.vector.tensor_tensor(cS[:, slot, :], Gs[:, slot, 64:128],
                                            cS[:, slot, :], ALU.mult)
                    nc.vector.tensor_tensor(cS[:, slot, :], cS[:, slot, :],
                                            T1[:, slot, :], ALU.add)
                    nc.scalar.activation(tcS[:, slot, :], cS[:, slot, :], AF.Tanh)
                    eng2 = nc.gpsimd if use_gpsimd else nc.vector
                    eng2.tensor_tensor(hS[:, slot, :], Gs[:, slot, 192:256],
                                       tcS[:, slot, :], ALU.mult)
                    nc.vector.transpose(hT_dst, hS[:, slot, :])

                # ---- warmup: skew-2 wavefront (layer l handles t = tau - 2l,
                # so a cell's input comes from 2 ticks back and only the
                # self-recurrence is a 1-tick dependency) ----
                for tau in range(T + 4):
                    cur = hTab[tau % 2]
                    nxt = hTab[(tau + 1) % 2]
                    Zp = lps.tile([128, 3, 512], dt.float32, tag="Zp")
                    Gs = lrl.tile([128, 3, 256], dt.float32, tag="Gs")
                    T1 = lrl.tile([128, 3, 64], dt.float32, tag="T1w")
                    tcS = lrl.tile([128, 3, 64], dt.float32, tag="tcS")
                    hS = lrl.tile([128, 3, 64], dt.bfloat16, tag="hS")
                    for l in (2, 1, 0):
                        t = tau - 2 * l
                        if t < 0 or t >= T:
                            continue
                        wl = wmov_sb[l]
                        rhss = [wl[:, k, :] for k in range(4)]
                        if l == 0:
                            stats = [xT4a[:, t:BT:T], xT4b[:, t:BT:T],
                                     cur[:, 0, 0:32], cur[:, 0, 32:64]]
                        else:
                            # y_{l-1}(t) was produced at tick tau-2 -> lives
                            # in the buffer of parity (tau-1) == nxt
                            stats = [nxt[:, l - 1, 0:32], nxt[:, l - 1, 32:64],
                                     cur[:, l, 0:32], cur[:, l, 32:64]]
                        cell_tick(Zp, Gs, T1, tcS, hS, l, stats,
                                  rhss, nxt[:, l, :], use_gpsimd=True)

                # layer l processes t=T-1 at tau=T-1+2l, writing
                # hTab[(T+2l)%2] == hTab[T%2] for every layer.
                hfin = hTab[T % 2]
                h1fin = hfin
                nc.vector.tensor_copy(pT[:, 0, :, :], hfin[:, 2, :]
                                      .rearrange("p (k b) -> p k b", k=2))

                # ---- decode: 2-cell wavefront (emit cell1, cell0 per tick) ----
                # double-buffered h0 state
                hd = [lst.tile([128, 64], dt.bfloat16, tag=f"hd{i}",
                               name=f"hd{i}") for i in range(2)]
                # seed: cell0/cell1 carry states continue from warmup layers 0/1
                nc.vector.tensor_copy(hd[(0) % 2][:], hfin[:, 0, :])
                for tau in range(S):
                    cur = hd[tau % 2]
                    nxt = hd[(tau + 1) % 2]
                    Zp = lps.tile([128, 2, 512], dt.float32, tag="Zp")
                    Gs = lrl.tile([128, 2, 256], dt.float32, tag="Gsd")
                    T1 = lrl.tile([128, 2, 64], dt.float32, tag="T1d")
                    tcS = lrl.tile([128, 2, 64], dt.float32, tag="tcSd")
                    hS = lrl.tile([128, 2, 64], dt.bfloat16, tag="hSd")
                    w1_ = tau           # cell1 computes step w1_
                    if 1 <= w1_ <= S - 1:
                        if w1_ == 1:
                            h1prev = [h1fin[:, 1, 0:32], h1fin[:, 1, 32:64]]
                        else:
                            h1prev = [pT[:, w1_ - 1, 0, :], pT[:, w1_ - 1, 1, :]]
                        # h1prev is ready a step earlier than h0 -> put it
                        # first so its matmuls (with start=True) run early
                        stats = h1prev + [cur[:, 0:32], cur[:, 32:64]]
                        rhss = [wdec_sb[1][:, k, :] for k in (2, 3, 0, 1)]
                        cell_tick(Zp, Gs, T1, tcS, hS, 1, stats,
                                  rhss, pT[:, w1_, :, :].rearrange("p k b -> p (k b)"),
                                  split_sigma=True)
                    w0 = tau + 1        # cell0 computes step w0
                    if w0 <= S - 1:
                        stats = [cur[:, 0:32], cur[:, 32:64],
                                 pT[:, w0 - 1, 0, :], pT[:, w0 - 1, 1, :]]
                        rhss = [wdec_sb[0][:, k, :] for k in (2, 3, 0, 1)]
                        cell_tick(Zp, Gs, T1, tcS, hS, 0, stats,
                                  rhss, nxt[:], split_sigma=True)

            # ================= final attention over p =================
            with (
                tc.tile_pool(name="fin_roll", bufs=4) as frl,
                tc.tile_pool(name="fin_ps", bufs=2, space="PSUM") as fps,
            ):
                for b in range(BC):
                    ppps = fps.tile([S, 64], dt.float32, tag="ppps")
                    s2ps = fps.tile([S, S], dt.float32, tag="s2ps")
                    for kt in range(2):
                        pslice = pT[:, :, kt, b]   # [128, S] stride 64
                        nc.tensor.matmul(ppps[:], pslice, predw_sb[:, kt, :],
                                         start=(kt == 0), stop=(kt == 1))
                        nc.tensor.matmul(s2ps[:], pslice, pslice,
                                         start=(kt == 0), stop=(kt == 1))
                    expw = frl.tile([S, S], dt.bfloat16, tag="expw")
                    z2 = frl.tile([S, 1], dt.float32, tag="z2")
                    nc.scalar.activation(expw[:], s2ps[:], AF.Exp,
                                         scale=float(attn_scale),
                                         accum_out=z2[:])
                    ppsb = frl.tile([S, 64], dt.bfloat16, tag="ppsb")
                    nc.vector.tensor_copy(ppsb[:], ppps[:])
                    ops = fps.tile([S, 64], dt.float32, tag="ops")
                    nc.tensor.matmul(ops[:], expw[:], ppsb[:])
                    rz2 = frl.tile([S, 1], dt.float32, tag="rz2")
                    nc.vector.reciprocal(rz2[:], z2[:])
                    nc.vector.tensor_scalar(outf[:, b * NF:(b + 1) * NF], ops[:],
                                            rz2[:], None, ALU.mult)
                nc.sync.dma_start(
                    d_out[:].rearrange("b s f -> s b f"),
                    outf[:].rearrange("s (b f) -> s b f", b=BC))

    nc.compile()
    return nc


_cache = {}


def kernel(**inputs):
    x = np.asarray(inputs["inputs"])
    T = x.shape[1]
    S = 64
    attn_scale = float(np.asarray(inputs["attn_scale"]))
    ncores = x.shape[0] // BC

    shared, percore = build_host_tensors(inputs, T)
    key = (T, S, round(attn_scale, 9))
    if key not in _cache:
        _cache[key] = build_program(T, S, attn_scale)
    nc = _cache[key]

    in_maps = [dict(shared, **percore[c]) for c in range(ncores)]
    res = run_bass_kernel_spmd(nc, in_maps, list(range(ncores)))
    out = np.concatenate([res.results[c]["out"] for c in range(ncores)], axis=0)
    return np.ascontiguousarray(out.astype(np.float32))
